# revision 35
# baseline (speedup 1.0000x reference)
"""Trainium2 Bass kernel for PairwiseBilinear.

Math (reference):
    u1 = concat(u, 1)  # [B, L, 257]
    v1 = concat(v, 1)  # [B, L, 257]
    inter[b, l, o, v] = sum_u u1[b, l, u] * W[u, o, v]
    scores[b, m, l, o] = sum_v v1[b, m, v] * inter[b, l, o, v]

Shapes: u, v: [8, 512, 256] f32; W: [257, 32, 257] f32; out: [8, 512, 512, 32] f32.

Strategy: data-parallel over batch (8 batches -> 8 cores), W replicated.
Bias decomposition (keeps contractions at a clean 256 = 2x128):
    Wc  = W[:256, :, :256]            (core)
    Wub = W[256, :, :256]             (u-ones row)   -> added to interA as per-v bias
    Wvb = W[:256, :, 256]             (v-ones col)   -> interB[l, o] = u @ Wvb
    Wuv = W[256, :, 256]              (corner)       -> added to interB
    interA[l,o,v] = u[l] @ Wc[:,o,:] + Wub[o,:]
    scores[m,l,o] = v[m] @ interA[l,o,:] + interB[l,o]

On-chip layouts (per core / batch):
    interA: [v(256: 2 part-tiles), o*512 + l] bf16
    phase2: psum[m(128), (o-pair, l-chunk)] = vT.T @ interA, then VectorE
            adds interB broadcast while scattering into [m, (l, o)] staging,
            which DMAs out fully contiguous.
"""

import numpy as np
import ml_dtypes

from concourse import bass, tile, mybir, library_config
from concourse.bass_utils import run_bass_kernel_spmd

B, L, D, O = 8, 512, 256, 32
N_CORES = 8
BF16 = mybir.dt.bfloat16
F32 = mybir.dt.float32
NP_BF16 = ml_dtypes.bfloat16

# Filled by kernel() after each run; test harness reads exec_time_ns from here.
LAST_RESULT = None

# walrus ISA structs have a single sync-wait slot.
_WAIT_LIMITS = {}
_DEFAULT_WAIT_LIMIT = 1


def _strip_redundant_waits(nc):
    """Drop own-engine semaphore waits from instructions that exceed the
    walrus per-instruction wait cap. Engines execute their queue strictly
    in order, so a wait on the engine's own completion semaphore (whose
    value necessarily refers to earlier instructions in the same queue) is
    always satisfied by the time the instruction issues."""
    import bass_rust

    f = nc.m.functions[0]
    for bb in f.blocks:
        # Split multi-wait Drains into a chain of single-wait Drains (the
        # CTRL struct also has a single wait slot).
        insts = bb.instructions
        for drain in [i for i in insts if i.__class__.__name__ == "InstDrain"]:
            si = drain.sync_info
            ow = list(si.on_wait) if si and si.on_wait else []
            if len(ow) <= 1:
                continue
            idx = [i.name for i in bb.instructions].index(drain.name)
            for ji, w in enumerate(ow[:-1]):
                d2 = type(drain)(name=f"{drain.name}_split{ji}", ins=[], outs=[])
                d2.engine = drain.engine
                d2.sync_info = bass_rust.SyncInfo(on_wait=[w], on_update=[])
                bb.instructions.insert(idx + ji, d2)
            si.on_wait = [ow[-1]]
        for inst in bb.instructions:
            k = inst.__class__.__name__
            if k in ("InstDrain", "InstEventSemaphore"):
                continue
            si = getattr(inst, "sync_info", None)
            ow = getattr(si, "on_wait", None) if si else None
            if not ow:
                continue
            limit = _WAIT_LIMITS.get(k, _DEFAULT_WAIT_LIMIT)
            if len(ow) <= limit:
                continue
            eng = getattr(inst, "engine", None)
            own = f"{eng.value}_" if eng is not None else None
            kept = [w for w in ow if not (own and w.ant_name.startswith(own))]
            if k == "InstDMACopy" and len(kept) > limit:
                # HWDGE DMAs issued from the same engine drain their ring in
                # FIFO order, and this kernel's out-DMAs are gated (via the
                # DVE->PE chain) far behind every input DMA, so the lane-
                # bookkeeping waits are redundant.
                kept = [w for w in kept if not w.ant_name.startswith("DMAHW")]
            assert len(kept) <= limit, (
                f"{inst.name} ({k}): still {len(kept)} waits after stripping "
                f"own-engine waits: {[w.ant_name for w in kept]}"
            )
            si.on_wait = kept


def _build_program():
    nc = bass.Bass("TRN2", target_bir_lowering=False, debug=False)

    uT_d = nc.dram_tensor("uT", [D, L], BF16, kind="ExternalInput").ap()
    vT_d = nc.dram_tensor("vT", [D, L], BF16, kind="ExternalInput").ap()
    # wc[o*256 + u, v] = W[u, o, v]
    wc_d = nc.dram_tensor("wc", [O * D, D], BF16, kind="ExternalInput").ap()
    # wub[v, o] = W[256, o, v]
    wub_d = nc.dram_tensor("wub", [D, O], F32, kind="ExternalInput").ap()
    # wvb[u, o] = W[u, o, 256]
    wvb_d = nc.dram_tensor("wvb", [D, O], BF16, kind="ExternalInput").ap()
    # wuv[o] = W[256, o, 256]
    wuv_d = nc.dram_tensor("wuv", [O, 1], F32, kind="ExternalInput").ap()
    # msk[k, o*128 + p] = 1.0 if k == o else 0.0  (selector for row-broadcast)
    msk_d = nc.dram_tensor("msk", [O, O * 128], BF16, kind="ExternalInput").ap()
    out_d = nc.dram_tensor("out", [L, L, O], F32, kind="ExternalOutput").ap()

    LC = 128          # l-chunk for phase-2 output staging
    N_LC = L // LC    # 4
    OP = 4            # o's per phase-2 matmul (N = OP * LC = 512)

    with tile.TileContext(nc) as tc:
        with (
            tc.tile_pool(name="const", bufs=1) as cpool,
            tc.tile_pool(name="ia", bufs=1) as iapool,
            tc.tile_pool(name="osb", bufs=2) as opool,
            tc.tile_pool(name="p1", bufs=4, space="PSUM") as p1pool,
            tc.tile_pool(name="p2", bufs=4, space="PSUM") as p2pool,
        ):
            # ---- constant loads ----
            ut = [cpool.tile([128, L], BF16, tag=f"ut{h}", name=f"ut{h}") for h in range(2)]
            vt = [cpool.tile([128, L], BF16, tag=f"vt{h}", name=f"vt{h}") for h in range(2)]
            wub = [cpool.tile([128, O], F32, tag=f"wub{h}", name=f"wub{h}") for h in range(2)]
            wvb = [cpool.tile([128, O], BF16, tag=f"wvb{h}", name=f"wvb{h}") for h in range(2)]
            wuv = cpool.tile([O, 1], F32, tag="wuv")
            msk = cpool.tile([O, O * 128], BF16, tag="msk")
            interb = cpool.tile([O, L], BF16, tag="interb")
            for h in range(2):
                sl = slice(h * 128, (h + 1) * 128)
                nc.sync.dma_start(out=ut[h][:], in_=uT_d[sl, :])
                nc.sync.dma_start(out=vt[h][:], in_=vT_d[sl, :])
                nc.sync.dma_start(out=wub[h][:], in_=wub_d[sl, :])
                nc.sync.dma_start(out=wvb[h][:], in_=wvb_d[sl, :])
            nc.sync.dma_start(out=wuv[:], in_=wuv_d[:])
            nc.sync.dma_start(out=msk[:], in_=msk_d[:])

            # ACT instructions support only ONE sync wait. Pre-sync the ACT
            # engine against the bias DMAs with dummy reads so the real
            # bias-adds below only wait on the PE semaphore.
            warm0 = cpool.tile([128, 1], F32, tag="warm0", name="warm0")
            warm1 = cpool.tile([128, 1], F32, tag="warm1", name="warm1")
            warm2 = cpool.tile([O, 1], F32, tag="warm2", name="warm2")
            nc.scalar.copy(out=warm0[:], in_=wub[0][:, 0:1])
            nc.scalar.copy(out=warm1[:], in_=wub[1][:, 0:1])
            nc.scalar.copy(out=warm2[:], in_=wuv[:])

            # all of Wc upfront in one 4MB DMA: block b = o*2 + uh
            wcall = cpool.tile([128, 2 * O * D], BF16, tag="wcall", name="wcall")
            nc.sync.dma_start(
                out=wcall[:].rearrange("p (b c) -> p b c", c=D),
                in_=wc_d.rearrange("(b p) c -> p b c", p=128),
            )
            wca3 = wcall[:].rearrange("p (b c) -> p b c", c=D)

            # PE sync sponges: 1-column ldweights reading each DMA-loaded
            # tensor, so real matmuls never carry a DMA wait (walrus allows
            # only one sync wait per compute instruction).
            for t in (ut[0], ut[1], vt[0], vt[1], wvb[0], wvb[1]):
                nc.tensor.ldweights(t[:, 0:1])
            nc.tensor.ldweights(msk[:, 0:1])
            nc.tensor.ldweights(wcall[:, 0:1])

            # ---- interB[o, l] = u @ Wvb + Wuv ----
            pB = p2pool.tile([O, L], F32, tag="p2")
            nc.tensor.matmul(pB[:], lhsT=wvb[0][:], rhs=ut[0][:], start=True, stop=False)
            nc.tensor.matmul(pB[:], lhsT=wvb[1][:], rhs=ut[1][:], start=False, stop=True)
            nc.scalar.add(interb[:], pB[:], add=wuv[:])

            # interA tiles: [128 (v half), O * L] with column index o*512 + l
            ia = [iapool.tile([128, O * L], BF16, tag=f"ia{h}", name=f"ia{h}") for h in range(2)]
            # interB broadcast to all partitions, same column layout
            bcast = iapool.tile([128, O * L], BF16, tag="bc")

            # ---- phase 1: interA[v, o*L + l] = (u @ Wc[:, o, :]).T + Wub ----
            for o in range(O):
                for vh in range(2):
                    p1 = p1pool.tile([128, L], F32, tag="p1")
                    vsl = slice(vh * 128, (vh + 1) * 128)
                    nc.tensor.matmul(
                        p1[:], lhsT=wca3[:, 2 * o, vsl], rhs=ut[0][:], start=True, stop=False
                    )
                    nc.tensor.matmul(
                        p1[:], lhsT=wca3[:, 2 * o + 1, vsl], rhs=ut[1][:], start=False, stop=True
                    )
                    nc.scalar.add(
                        ia[vh][:, o * L : (o + 1) * L], p1[:], add=wub[vh][:, o : o + 1]
                    )
                # broadcast interb[o, :] to all 128 partitions via one-hot matmul
                pbc = p1pool.tile([128, L], F32, tag="p1", name=f"pbc{o}")
                nc.tensor.matmul(
                    pbc[:],
                    lhsT=msk[:, o * 128 : (o + 1) * 128],
                    rhs=interb[:],
                    start=True,
                    stop=True,
                )
                # on DVE so phase-2 TTs see bcast via same-engine FIFO order
                nc.vector.tensor_copy(out=bcast[:, o * L : (o + 1) * L], in_=pbc[:])

            # PE sponge: observe the last interA ACT writes so phase-2
            # matmuls only wait on the DVE (psum-slot) semaphore.
            nc.tensor.ldweights(ia[0][0:1, O * L - 1 : O * L])
            nc.tensor.ldweights(ia[1][0:1, O * L - 1 : O * L])

            # ---- phase 2: scores[m, l, o] = vT.T @ interA + bcast ----
            ia3 = [t[:].rearrange("p (o l) -> p o l", o=O) for t in ia]
            bc3 = bcast[:].rearrange("p (o l) -> p o l", o=O)
            for m in range(L // 128):
                msl = slice(m * 128, (m + 1) * 128)
                for lc in range(N_LC):
                    lsl = slice(lc * LC, (lc + 1) * LC)
                    osb = opool.tile([128, LC * O], F32, tag="osb")
                    # DVE sponge: memset absorbs the osb slot-reuse DMA wait
                    # so the TTs below only carry the PE wait.
                    nc.vector.memset(osb[0:1, 0:1], 0.0)
                    osb3 = osb[:].rearrange("p (l o) -> p o l", o=O)
                    for j in range(O // OP):
                        osl = slice(j * OP, (j + 1) * OP)
                        p2 = p2pool.tile([128, OP, LC], F32, tag="p2")
                        nc.tensor.matmul(
                            p2[:],
                            lhsT=vt[0][:, msl],
                            rhs=ia3[0][:, osl, lsl],
                            start=True,
                            stop=False,
                        )
                        nc.tensor.matmul(
                            p2[:],
                            lhsT=vt[1][:, msl],
                            rhs=ia3[1][:, osl, lsl],
                            start=False,
                            stop=True,
                        )
                        nc.vector.tensor_tensor(
                            out=osb3[:, osl, :],
                            in0=p2[:],
                            in1=bc3[:, osl, lsl],
                            op=mybir.AluOpType.add,
                        )
                    nc.sync.dma_start(
                        out=out_d[msl, lsl, :],
                        in_=osb[:].rearrange("p (l o) -> p l o", o=O),
                    )
    _strip_redundant_waits(nc)
    return nc


def kernel(u, v, weight):
    global LAST_RESULT
    u = np.asarray(u, dtype=np.float32)
    v = np.asarray(v, dtype=np.float32)
    w = np.asarray(weight, dtype=np.float32)
    assert u.shape == (B, L, D) and v.shape == (B, L, D)
    assert w.shape == (D + 1, O, D + 1)

    # Host-side packing (layout only; counted work happens on device).
    uT = np.ascontiguousarray(u.transpose(0, 2, 1)).astype(NP_BF16)  # [B, D, L]
    vT = np.ascontiguousarray(v.transpose(0, 2, 1)).astype(NP_BF16)
    wc = (
        np.ascontiguousarray(w[:D, :, :D].transpose(1, 0, 2))
        .reshape(O * D, D)
        .astype(NP_BF16)
    )
    wub = np.ascontiguousarray(w[D, :, :D].T).astype(np.float32)  # [D, O]
    wvb = np.ascontiguousarray(w[:D, :, D]).astype(NP_BF16)  # [D, O]
    wuv = np.ascontiguousarray(w[D, :, D].reshape(O, 1)).astype(np.float32)
    msk = np.zeros((O, O * 128), dtype=NP_BF16)
    for o in range(O):
        msk[o, o * 128 : (o + 1) * 128] = 1.0

    nc = _build_program()
    in_maps = [
        {
            "uT": uT[i],
            "vT": vT[i],
            "wc": wc,
            "wub": wub,
            "wvb": wvb,
            "wuv": wuv,
            "msk": msk,
        }
        for i in range(N_CORES)
    ]
    LAST_RESULT = run_bass_kernel_spmd(nc, in_maps, list(range(N_CORES)))
    out = np.stack([LAST_RESULT.results[i]["out"] for i in range(N_CORES)], axis=0)
    return out.astype(np.float32)
